# revision 1
# baseline (speedup 1.0000x reference)
"""Trainium2 Bass kernel for the L1Writer scatter-memory problem.

Computes   out = 0.95 * memory + einsum('bs,bshk,bshv->hkv', rho, keys, values)

Strategy: data-parallel over the flattened (B*S)=16384 token axis, 2048 rows
per core.  Each core computes its partial delta
    delta_h = K_h^T diag(rho) V_h        (per head h, shapes (2048,64))
as a chain of 128-row PE matmuls accumulating in PSUM.  The 8 partial
(H,Dk,Dv) deltas are summed on the host (tiny: 256 KB each) and added to
decay*memory there.

Per-core kernel layout:
  - keys/values arrive as (2048, 1024) row-major shards; loaded in 4 mega
    tiles of [128 partitions x 4096 fp32] (2 MB DMAs, 4 KB contiguous runs).
  - rho arrives pre-transposed as (128, 16): partition p, chunk c -> rho of
    token c*128+p.  Keys are scaled by rho on the vector engine
    (per-partition tensor_scalar broadcast).
  - 16 heads accumulate into 2 PSUM banks ([64, 512] each, 8 heads per
    bank).  Banks are zeroed with a DVE memset and every matmul uses
    start=False, so each element's first matmul overwrites (has_written
    unset) or accumulates onto the memset zero (has_written stale-set);
    both give the correct sum without any whole-bank-clear hazards.
  - PSUM -> SBUF copy -> one contiguous 256 KB DMA out in [k, h*64+v]
    layout; the host transposes to (h, k, v).
"""

import numpy as np

DECAY = 0.95
B, S, H, Dk, Dv = 4, 4096, 16, 64, 64
N_CORES = 8
NS = (B * S) // N_CORES          # 2048 rows per core
P = 128                          # partitions
CHUNKS = NS // P                 # 16 contraction chunks of 128 rows
MEGA = 4                         # chunks per DMA mega-tile
N_MEGA = CHUNKS // MEGA          # 4 mega tiles
FD = H * Dk                      # 1024 features per row

_nc_cache = None


def _build_nc():
    from contextlib import ExitStack

    import concourse.bass as bass
    import concourse.mybir as mybir

    f32 = mybir.dt.float32
    nc = bass.Bass()

    keys_d = nc.dram_tensor("keys", (NS, FD), f32, kind="ExternalInput")
    vals_d = nc.dram_tensor("values", (NS, FD), f32, kind="ExternalInput")
    rho_d = nc.dram_tensor("rho", (P, CHUNKS), f32, kind="ExternalInput")
    out_d = nc.dram_tensor("delta", (Dk, H * Dv), f32, kind="ExternalOutput")

    # mega tile m, partition p, free (j, f): row (m*MEGA + j)*128 + p
    keys_r = keys_d.rearrange("(m j p) f -> m p j f", j=MEGA, p=P)
    vals_r = vals_d.rearrange("(m j p) f -> m p j f", j=MEGA, p=P)

    # Raw bass (no Tile): this container's walrus rejects engine
    # instructions carrying >1 attached semaphore wait, so all waits are
    # standalone sequencer wait_ge ops and every hazard is hand-managed.
    #
    # Engine programs:
    #  SP (sync):  rho DMA, then kt[m]/vt[m] mega DMAs (2 MB each,
    #              double-buffered; WAR waits vs DVE/PE for slot reuse),
    #              final out DMA.
    #  DVE:        memset both PSUM accumulators, then per (m,j) scale keys
    #              by rho (per-partition tensor_scalar), finally evacuate
    #              PSUM -> SBUF.
    #  PE:         per (m,j): 16 head matmuls accumulating into 2 PSUM
    #              banks (8 heads x 64 cols each); all start=False onto
    #              memset zeros (first write per element overwrites or
    #              adds to zero -- correct for any stale has_written bits).
    #
    # dve_sem increments: 1 (memsets) + 16 (scales) + 2 (evac) = 19
    # pe_sem increments: 1 per (m,j) group = 16
    with ExitStack() as ctx:
        kt = [
            ctx.enter_context(nc.sbuf_tensor(f"kt{i}", [P, MEGA, FD], f32))
            for i in range(2)
        ]
        vt = [
            ctx.enter_context(nc.sbuf_tensor(f"vt{i}", [P, MEGA, FD], f32))
            for i in range(2)
        ]
        kts = [
            ctx.enter_context(nc.sbuf_tensor(f"kts{i}", [P, MEGA, FD], f32))
            for i in range(2)
        ]
        rho_t = ctx.enter_context(nc.sbuf_tensor("rho_t", [P, CHUNKS], f32))
        out_t = ctx.enter_context(nc.sbuf_tensor("out_t", [Dk, H * Dv], f32))
        acc = [
            ctx.enter_context(nc.psum_tensor(f"acc{i}", [Dk, 8 * Dv], f32))
            for i in range(2)
        ]
        rs = ctx.enter_context(nc.semaphore(name="rs"))
        ks = [ctx.enter_context(nc.semaphore(name=f"ks{i}")) for i in range(N_MEGA)]
        vs = [ctx.enter_context(nc.semaphore(name=f"vs{i}")) for i in range(N_MEGA)]
        dve_sem = ctx.enter_context(nc.semaphore(name="dve_sem"))
        out_sem = ctx.enter_context(nc.semaphore(name="out_sem"))
        done_sem = ctx.enter_context(nc.semaphore(name="done_sem"))
        pe_sem = ctx.enter_context(nc.semaphore(name="pe_sem"))
        block = ctx.enter_context(nc.Block())

        @block.sync
        def _(sync):
            sync.dma_start(rho_t[:], rho_d[:]).then_inc(rs, 16)
            for m in range(N_MEGA):
                if m >= 2:
                    # kt slot WAR: scales of m-2 done (1 + (m-2)*4 + 4)
                    sync.wait_ge(dve_sem, (m - 2) * 4 + 5)
                sync.dma_start(kt[m % 2][:], keys_r[m]).then_inc(ks[m], 16)
                if m >= 2:
                    # vt slot WAR: matmul groups of m-2 done
                    sync.wait_ge(pe_sem, (m - 2) * 4 + 4)
                sync.dma_start(vt[m % 2][:], vals_r[m]).then_inc(vs[m], 16)
            sync.wait_ge(dve_sem, 19)
            sync.dma_start(out_d[:], out_t[:]).then_inc(out_sem, 16)
            sync.wait_ge(out_sem, 16)
            sync.nop().then_inc(done_sem, 1)

        @block.gpsimd
        def _(gpsimd):
            # Semaphores persist across NEFF executions; clear them all at
            # the end (after every engine is provably done) so the kernel
            # is safe to run repeatedly.
            gpsimd.wait_ge(done_sem, 1)
            for s in [rs, *ks, *vs, dve_sem, pe_sem, out_sem, done_sem]:
                gpsimd.sem_clear(s)

        @block.vector
        def _(vector):
            vector.memset(acc[0][:], 0.0)
            vector.memset(acc[1][:], 0.0).then_inc(dve_sem, 1)
            vector.wait_ge(rs, 16)
            for m in range(N_MEGA):
                vector.wait_ge(ks[m], 16)
                if m >= 2:
                    # kts slot WAR: matmul groups of m-2 done
                    vector.wait_ge(pe_sem, (m - 2) * 4 + 4)
                for j in range(MEGA):
                    c = m * MEGA + j
                    vector.tensor_scalar_mul(
                        kts[m % 2][:, j, :],
                        kt[m % 2][:, j, :],
                        rho_t[:, c : c + 1],
                    ).then_inc(dve_sem, 1)
            vector.wait_ge(pe_sem, 16)
            for g in range(2):
                vector.tensor_copy(
                    out_t[:, g * 512 : (g + 1) * 512], acc[g][:]
                ).then_inc(dve_sem, 1)

        @block.tensor
        def _(tensor):
            for m in range(N_MEGA):
                tensor.wait_ge(vs[m], 16)
                for j in range(MEGA):
                    # memsets + scales up to (m,j) done
                    tensor.wait_ge(dve_sem, m * 4 + j + 2)
                    for h in range(H):
                        g, hh = divmod(h, 8)
                        mm = tensor.matmul(
                            acc[g][:, hh * Dv : (hh + 1) * Dv],
                            kts[m % 2][:, j, h * Dk : (h + 1) * Dk],
                            vt[m % 2][:, j, h * Dv : (h + 1) * Dv],
                            start=False,
                            stop=(m == N_MEGA - 1 and j == MEGA - 1),
                            skip_group_check=True,
                        )
                        if h == H - 1:
                            mm.then_inc(pe_sem, 1)

    return nc


def _get_nc():
    global _nc_cache
    if _nc_cache is None:
        _nc_cache = _build_nc()
    return _nc_cache


def _make_in_maps(keys, values, write_strengths):
    kf = np.ascontiguousarray(keys.reshape(B * S, FD))
    vf = np.ascontiguousarray(values.reshape(B * S, FD))
    wf = np.asarray(write_strengths).reshape(B * S)
    in_maps = []
    for c in range(N_CORES):
        sl = slice(c * NS, (c + 1) * NS)
        in_maps.append(
            {
                "keys": np.ascontiguousarray(kf[sl]),
                "values": np.ascontiguousarray(vf[sl]),
                "rho": np.ascontiguousarray(wf[sl].reshape(CHUNKS, P).T),
            }
        )
    return in_maps


def _run(in_maps, **kwargs):
    from concourse.bass_utils import run_bass_kernel_spmd

    nc = _get_nc()
    return run_bass_kernel_spmd(nc, in_maps, core_ids=list(range(N_CORES)), **kwargs)


def _assemble(memory, results):
    parts = np.stack([r["delta"] for r in results], axis=0)  # (8, 64, 1024)
    delta = parts.sum(axis=0, dtype=np.float64)  # (64, 1024) in [k, h*64+v]
    delta_hkv = delta.reshape(Dk, H, Dv).transpose(1, 0, 2)  # (H, Dk, Dv)
    out = DECAY * np.asarray(memory, dtype=np.float64) + delta_hkv
    return out.astype(np.float32)


def kernel(memory, keys, values, write_strengths):
    memory = np.asarray(memory, dtype=np.float32)
    keys = np.asarray(keys, dtype=np.float32)
    values = np.asarray(values, dtype=np.float32)
    write_strengths = np.asarray(write_strengths, dtype=np.float32)

    in_maps = _make_in_maps(keys, values, write_strengths)
    res = _run(in_maps)
    return _assemble(memory, res.results)


if __name__ == "__main__":
    rng = np.random.default_rng(0)
    mem = rng.standard_normal((H, Dk, Dv), dtype=np.float32)
    k = rng.standard_normal((B, S, H, Dk), dtype=np.float32)
    v = rng.standard_normal((B, S, H, Dv), dtype=np.float32)
    w = rng.random((B, S), dtype=np.float32)
    out = kernel(mem, k, v, w)
    ref = DECAY * mem + np.einsum(
        "bs,bshk,bshv->hkv", w.astype(np.float64), k.astype(np.float64), v.astype(np.float64)
    )
    err = np.abs(out - ref).max() / np.abs(ref).max()
    print("self-check rel err:", err)



# revision 5
# speedup vs baseline: 1.6382x; 1.6382x over previous
"""Trainium2 Bass kernel for the L1Writer scatter-memory problem.

Computes   out = 0.95 * memory + einsum('bs,bshk,bshv->hkv', rho, keys, values)

Strategy: data-parallel over the flattened (B*S)=16384 token axis, 2048 rows
per core.  Each core computes its partial delta
    delta_h = K_h^T diag(rho) V_h        (per head h, shapes (2048,64))
as a chain of 128-row PE matmuls accumulating in PSUM.  The 8 partial
(H,Dk,Dv) deltas are summed on the host (tiny: 256 KB each) and added to
decay*memory there.

Keys/values are cast to bf16 on the host (PSUM accumulation stays fp32;
measured end-to-end rel err ~2e-3, well inside the 2e-2 gate).  This halves
HBM traffic — the kernel is memory-bound, so it is ~2x faster than fp32 —
and bf16 matmuls run single-pass (fp32 matmuls are split into HI/LO pairs).

Per-core kernel layout:
  - keys/values arrive as (128, 16, 1024) bf16, host-transposed so that
    token c*128+p lives at [p, c, :]: every DMA partition line is one
    contiguous 8 KB run in DRAM.  Loaded as 4 mega tiles of
    [128, 4, 1024] (1 MB DMAs), kt/vt interleaved so the rho-scale and
    the matmuls pipeline with the remaining loads.  Everything fits in
    SBUF at once (12 MB), so there is no buffer reuse and no WAR waits.
  - rho arrives pre-transposed as (128, 16) bf16: [p, c] = rho of token
    c*128+p.  Keys are scaled by rho on the vector engine (per-partition
    tensor_scalar broadcast, bf16 2x mode).
  - 16 heads accumulate into 2 PSUM banks ([64, 512] each, 8 heads per
    bank).  Banks are zeroed with a DVE memset and every matmul uses
    start=False, so each element's first matmul overwrites (has_written
    unset) or accumulates onto the memset zero (has_written stale-set);
    both give the correct sum without any whole-bank-clear hazards.
  - PSUM -> SBUF copy -> one contiguous 256 KB fp32 DMA out in
    [k, h*64+v] layout; the host transposes to (h, k, v).
"""

import numpy as np
import ml_dtypes

BF16 = ml_dtypes.bfloat16

DECAY = 0.95
B, S, H, Dk, Dv = 4, 4096, 16, 64, 64
N_CORES = 8
NS = (B * S) // N_CORES          # 2048 rows per core
P = 128                          # partitions
CHUNKS = NS // P                 # 16 contraction chunks of 128 rows
MEGA = 4                         # chunks per DMA mega-tile
N_MEGA = CHUNKS // MEGA          # 4 mega tiles
FD = H * Dk                      # 1024 features per row

_nc_cache = None


def _build_nc():
    from contextlib import ExitStack

    import concourse.bass as bass
    import concourse.mybir as mybir

    f32 = mybir.dt.float32
    bf16 = mybir.dt.bfloat16
    nc = bass.Bass()

    keys_d = nc.dram_tensor("keys", (P, CHUNKS, FD), bf16, kind="ExternalInput")
    vals_d = nc.dram_tensor("values", (P, CHUNKS, FD), bf16, kind="ExternalInput")
    rho_d = nc.dram_tensor("rho", (P, CHUNKS), f32, kind="ExternalInput")
    out_d = nc.dram_tensor("delta", (Dk, H * Dv), f32, kind="ExternalOutput")

    # Raw bass (no Tile): this container's walrus rejects engine
    # instructions carrying >1 attached semaphore wait, so all waits are
    # standalone sequencer wait_ge ops and every hazard is hand-managed.
    #
    # Engine programs:
    #  SP (sync):  rho DMA, then interleaved kt[m]/vt[m] mega DMAs (1 MB
    #              each, all queued up front -- no reuse hazards), final
    #              out DMA.
    #  DVE:        memset both PSUM accumulators, then per (m,j) scale keys
    #              by rho (per-partition tensor_scalar), finally evacuate
    #              PSUM -> SBUF.
    #  PE:         per (m,j): 16 head matmuls accumulating into 2 PSUM
    #              banks (8 heads x 64 cols each); all start=False onto
    #              memset zeros (first write per element overwrites or
    #              adds to zero -- correct for any stale has_written bits).
    #
    # dve_sem increments: 1 (memsets) + 16 (scales) + 2 (evac) = 19
    # pe_sem increments: 1 per (m,j) group = 16
    with ExitStack() as ctx:
        kt = ctx.enter_context(nc.sbuf_tensor("kt", [P, CHUNKS, FD], bf16))
        kts = ctx.enter_context(nc.sbuf_tensor("kts", [P, CHUNKS, FD], bf16))
        vt = ctx.enter_context(nc.sbuf_tensor("vt", [P, CHUNKS, FD], bf16))
        rho_t = ctx.enter_context(nc.sbuf_tensor("rho_t", [P, CHUNKS], f32))
        out_t = ctx.enter_context(nc.sbuf_tensor("out_t", [Dk, H * Dv], f32))
        acc = [
            ctx.enter_context(nc.psum_tensor(f"acc{i}", [Dk, 8 * Dv], f32))
            for i in range(2)
        ]
        rs = ctx.enter_context(nc.semaphore(name="rs"))
        ks = [ctx.enter_context(nc.semaphore(name=f"ks{i}")) for i in range(N_MEGA)]
        vs = [ctx.enter_context(nc.semaphore(name=f"vs{i}")) for i in range(N_MEGA)]
        dve_sem = ctx.enter_context(nc.semaphore(name="dve_sem"))
        out_sem = ctx.enter_context(nc.semaphore(name="out_sem"))
        done_sem = ctx.enter_context(nc.semaphore(name="done_sem"))
        pe_sem = ctx.enter_context(nc.semaphore(name="pe_sem"))
        block = ctx.enter_context(nc.Block())

        @block.sync
        def _(sync):
            sync.dma_start(rho_t[:], rho_d[:]).then_inc(rs, 16)
            for m in range(N_MEGA):
                sl = slice(m * MEGA, (m + 1) * MEGA)
                sync.dma_start(kt[:, sl, :], keys_d[:, sl, :]).then_inc(ks[m], 16)
                sync.dma_start(vt[:, sl, :], vals_d[:, sl, :]).then_inc(vs[m], 16)
            sync.wait_ge(dve_sem, 19)
            sync.dma_start(out_d[:], out_t[:]).then_inc(out_sem, 16)
            sync.wait_ge(out_sem, 16)
            sync.nop().then_inc(done_sem, 1)

        @block.gpsimd
        def _(gpsimd):
            # Semaphores persist across NEFF executions; clear them all at
            # the end (after every engine is provably done) so the kernel
            # is safe to run repeatedly.
            gpsimd.wait_ge(done_sem, 1)
            for s in [rs, *ks, *vs, dve_sem, pe_sem, out_sem, done_sem]:
                gpsimd.sem_clear(s)

        @block.vector
        def _(vector):
            vector.memset(acc[0][:], 0.0)
            vector.memset(acc[1][:], 0.0).then_inc(dve_sem, 1)
            vector.wait_ge(rs, 16)
            for m in range(N_MEGA):
                vector.wait_ge(ks[m], 16)
                for j in range(MEGA):
                    c = m * MEGA + j
                    vector.tensor_scalar_mul(
                        kts[:, c, :],
                        kt[:, c, :],
                        rho_t[:, c : c + 1],
                    ).then_inc(dve_sem, 1)
            vector.wait_ge(pe_sem, 16)
            for g in range(2):
                vector.tensor_copy(
                    out_t[:, g * 512 : (g + 1) * 512], acc[g][:]
                ).then_inc(dve_sem, 1)

        @block.tensor
        def _(tensor):
            for m in range(N_MEGA):
                tensor.wait_ge(vs[m], 16)
                for j in range(MEGA):
                    c = m * MEGA + j
                    # memsets + scales up to (m,j) done
                    tensor.wait_ge(dve_sem, c + 2)
                    for h in range(H):
                        g, hh = divmod(h, 8)
                        mm = tensor.matmul(
                            acc[g][:, hh * Dv : (hh + 1) * Dv],
                            kts[:, c, h * Dk : (h + 1) * Dk],
                            vt[:, c, h * Dv : (h + 1) * Dv],
                            start=False,
                            stop=(m == N_MEGA - 1 and j == MEGA - 1),
                            skip_group_check=True,
                        )
                        if h == H - 1:
                            mm.then_inc(pe_sem, 1)

    return nc


def _get_nc():
    global _nc_cache
    if _nc_cache is None:
        _nc_cache = _build_nc()
    return _nc_cache


def _make_in_maps(keys, values, write_strengths):
    # One full-array bf16 cast, then per-core (chunk, p, f) -> (p, chunk, f)
    # transposes so each DMA partition line is contiguous in DRAM.
    kf = keys.reshape(B * S, FD).astype(BF16)
    vf = values.reshape(B * S, FD).astype(BF16)
    wf = np.asarray(write_strengths).reshape(B * S).astype(np.float32)
    in_maps = []
    for c in range(N_CORES):
        sl = slice(c * NS, (c + 1) * NS)
        in_maps.append(
            {
                "keys": np.ascontiguousarray(
                    kf[sl].reshape(CHUNKS, P, FD).transpose(1, 0, 2)
                ),
                "values": np.ascontiguousarray(
                    vf[sl].reshape(CHUNKS, P, FD).transpose(1, 0, 2)
                ),
                "rho": np.ascontiguousarray(wf[sl].reshape(CHUNKS, P).T),
            }
        )
    return in_maps


def _run(in_maps, **kwargs):
    from concourse.bass_utils import run_bass_kernel_spmd

    nc = _get_nc()
    return run_bass_kernel_spmd(nc, in_maps, core_ids=list(range(N_CORES)), **kwargs)


def _assemble(memory, results):
    parts = np.stack([r["delta"] for r in results], axis=0)  # (8, 64, 1024)
    delta = parts.sum(axis=0, dtype=np.float64)  # (64, 1024) in [k, h*64+v]
    delta_hkv = delta.reshape(Dk, H, Dv).transpose(1, 0, 2)  # (H, Dk, Dv)
    out = DECAY * np.asarray(memory, dtype=np.float64) + delta_hkv
    return out.astype(np.float32)


def kernel(memory, keys, values, write_strengths):
    memory = np.asarray(memory, dtype=np.float32)
    keys = np.asarray(keys, dtype=np.float32)
    values = np.asarray(values, dtype=np.float32)
    write_strengths = np.asarray(write_strengths, dtype=np.float32)

    in_maps = _make_in_maps(keys, values, write_strengths)
    res = _run(in_maps)
    return _assemble(memory, res.results)


if __name__ == "__main__":
    rng = np.random.default_rng(0)
    mem = rng.standard_normal((H, Dk, Dv), dtype=np.float32)
    k = rng.standard_normal((B, S, H, Dk), dtype=np.float32)
    v = rng.standard_normal((B, S, H, Dv), dtype=np.float32)
    w = rng.random((B, S), dtype=np.float32)
    out = kernel(mem, k, v, w)
    ref = DECAY * mem + np.einsum(
        "bs,bshk,bshv->hkv", w.astype(np.float64), k.astype(np.float64), v.astype(np.float64)
    )
    err = np.abs(out - ref).max() / np.abs(ref).max()
    print("self-check rel err:", err)


# revision 24
# speedup vs baseline: 1.7856x; 1.0900x over previous
"""Trainium2 Bass kernel for the L1Writer scatter-memory problem.

Computes   out = 0.95 * memory + einsum('bs,bshk,bshv->hkv', rho, keys, values)

Strategy: data-parallel over the flattened (B*S)=16384 token axis, 2048 rows
per core.  Each core computes its partial delta
    delta_h = K_h^T diag(rho) V_h        (per head h, shapes (2048,64))
as a chain of 128-row PE matmuls accumulating in PSUM.  The 8 partial
(H,Dk,Dv) deltas are summed on the host (tiny: 256 KB each) and added to
decay*memory there.

Keys/values are cast to bf16 on the host (PSUM accumulation stays fp32;
measured end-to-end rel err ~6e-4, well inside the 2e-2 gate).  This halves
HBM traffic — the kernel is memory-bound, so it is ~2x faster than fp32 —
and bf16 matmuls run single-pass (fp32 matmuls are split into HI/LO pairs).

Per-core kernel layout:
  - keys/values arrive as (128, 16, 1024) bf16, host-transposed so that
    token c*128+p lives at [p, c, :]: every DMA partition line is one
    contiguous run in DRAM.  Keys load as 4 mega tiles of [128, 4, 1024]
    (1 MB DMAs); values load per chunk ([128, 1, 1024], 256 KB DMAs) so
    the tensor engine's last wait covers only 1/16 of the stream — the
    post-DMA tail is 16 matmuls, not 64.  Everything fits in SBUF at
    once (12 MB), so there is no buffer reuse and no WAR waits.
  - rho arrives pre-transposed as (128, 16) fp32: [p, c] = rho of token
    c*128+p.  Keys are scaled by rho on the vector engine (per-partition
    tensor_scalar broadcast); the scale of chunk c hides under the value
    DMAs of its own mega tile.
  - 16 heads accumulate into 2 PSUM banks ([64, 512] each, 8 heads per
    bank; separate banks by allocation).  Banks are zeroed with a DVE
    memset and every matmul uses start=False, so each element's first
    matmul overwrites (has_written unset) or accumulates onto the memset
    zero (has_written stale-set); both give the correct sum without any
    whole-bank-clear hazards.
  - The last chunk runs bank-0 heads first, so the scalar engine copies
    bank 0 to SBUF and DMAs it to DRAM rows 0-63 (ACT is an HWDGE
    engine) while the tensor engine finishes bank 1 (different PSUM
    banks - no collision); the vector engine then copies bank 1 and the
    sync engine DMAs it to DRAM rows 64-127.  Both copies and the first
    output DMA hide under the remaining compute; the DRAM delta is
    (128, 512) fp32: rows 0-63 = heads 0-7, rows 64-127 = heads 8-15,
    [k, hh*64+v]; the host reassembles (H, Dk, Dv).
"""

import numpy as np
import ml_dtypes

BF16 = ml_dtypes.bfloat16

DECAY = 0.95
B, S, H, Dk, Dv = 4, 4096, 16, 64, 64
N_CORES = 8
NS = (B * S) // N_CORES          # 2048 rows per core
P = 128                          # partitions
CHUNKS = NS // P                 # 16 contraction chunks of 128 rows
MEGA = 4                         # chunks per keys DMA mega-tile
N_MEGA = CHUNKS // MEGA          # 4 mega tiles
FD = H * Dk                      # 1024 features per row

_nc_cache = None


def _build_nc():
    from contextlib import ExitStack

    import concourse.bass as bass
    import concourse.mybir as mybir

    f32 = mybir.dt.float32
    bf16 = mybir.dt.bfloat16
    nc = bass.Bass()

    keys_d = nc.dram_tensor("keys", (P, CHUNKS, FD), bf16, kind="ExternalInput")
    vals_d = nc.dram_tensor("values", (P, CHUNKS, FD), bf16, kind="ExternalInput")
    rho_d = nc.dram_tensor("rho", (P, CHUNKS), f32, kind="ExternalInput")
    out_d = nc.dram_tensor("delta", (P, 512), f32, kind="ExternalOutput")

    # Raw bass (no Tile): this container's walrus rejects engine
    # instructions carrying >1 attached semaphore wait, so all waits are
    # standalone sequencer wait_ge ops and every hazard is hand-managed.
    #
    # Engine programs:
    #  SP (sync):  rho DMA, then per mega tile: one 1 MB keys DMA followed
    #              by four 256 KB value-chunk DMAs (all queued up front --
    #              no reuse hazards), then the bank-1 half of the output.
    #  ACT:        issues the DRAM rows 0-63 output DMA once bank 0 is in
    #              SBUF (ACT is an HWDGE engine with its own ring, so the
    #              two output halves' DMAs overlap).  ACT does no datapath
    #              work: an ACT activation-copy feeding an ACT dma_start
    #              produced garbage (the sequencer-level DMA trigger does
    #              not order against the in-flight datapath op).
    #  DVE:        memset both PSUM accumulators, then per chunk scale keys
    #              by rho (per-partition tensor_scalar), finally evacuate
    #              PSUM bank 0 -> SBUF (overlapping PE's bank-1 heads --
    #              different banks, no collision) and then bank 1.
    #  PE:         per chunk: 16 head matmuls accumulating into 2 PSUM
    #              banks (8 heads x 64 cols each); all start=False onto
    #              memset zeros (first write per element overwrites or
    #              adds to zero -- correct for any stale has_written bits).
    #
    # Semaphores (all DMA incs are x16).  Every DMA gets its OWN semaphore:
    # a `then_inc(sem, 16)` is 16 independent +1s from the 16 SDMA engines,
    # so a shared counting semaphore across several DMAs is unsound -- the
    # count can reach 16*(c+1) with some engines a transfer ahead while
    # others have not finished transfer c (observed as nondeterminism).
    #  ks[m]: +16 for keys mega m -> scale of chunk c waits ks[c//4] >= 16
    #  vs[c]: +16 for value chunk c -> PE chunk c waits vs[c] >= 16
    #  dve_sem: 1 memset + 16 scales (PE chunk c waits c+2) + 1 bank-0
    #           evac copy (ACT's out DMA waits 18) + 1 bank-1 evac copy
    #           (sync's out DMA waits 19)
    #  pe_sem: +1 per chunk 0..14, +1 after last chunk's bank-0 heads
    #          (DVE bank-0 evac waits 16), +1 after bank-1 heads (DVE
    #          bank-1 evac waits 17)
    #  out_sem: +16 per output half; sync waits 32
    with ExitStack() as ctx:
        kt = ctx.enter_context(nc.sbuf_tensor("kt", [P, CHUNKS, FD], bf16))
        kts = ctx.enter_context(nc.sbuf_tensor("kts", [P, CHUNKS, FD], bf16))
        vt = ctx.enter_context(nc.sbuf_tensor("vt", [P, CHUNKS, FD], bf16))
        rho_t = ctx.enter_context(nc.sbuf_tensor("rho_t", [P, CHUNKS], f32))
        out_t0 = ctx.enter_context(nc.sbuf_tensor("out_t0", [Dk, 512], f32))
        out_t1 = ctx.enter_context(nc.sbuf_tensor("out_t1", [Dk, 512], f32))
        acc = [
            ctx.enter_context(nc.psum_tensor(f"acc{i}", [Dk, 8 * Dv], f32))
            for i in range(2)
        ]
        rs = ctx.enter_context(nc.semaphore(name="rs"))
        ks = [ctx.enter_context(nc.semaphore(name=f"ks{i}")) for i in range(N_MEGA)]
        vs = [ctx.enter_context(nc.semaphore(name=f"vs{i}")) for i in range(CHUNKS)]
        dve_sem = ctx.enter_context(nc.semaphore(name="dve_sem"))
        out_sem = ctx.enter_context(nc.semaphore(name="out_sem"))
        done_sem = ctx.enter_context(nc.semaphore(name="done_sem"))
        pe_sem = ctx.enter_context(nc.semaphore(name="pe_sem"))
        block = ctx.enter_context(nc.Block())

        @block.sync
        def _(sync):
            for m in range(N_MEGA):
                sl = slice(m * MEGA, (m + 1) * MEGA)
                sync.dma_start(kt[:, sl, :], keys_d[:, sl, :]).then_inc(ks[m], 16)
                if m == 0:
                    sync.dma_start(rho_t[:], rho_d[:]).then_inc(rs, 16)
                for j in range(MEGA):
                    c = m * MEGA + j
                    sync.dma_start(
                        vt[:, c : c + 1, :], vals_d[:, c : c + 1, :]
                    ).then_inc(vs[c], 16)
            sync.wait_ge(dve_sem, 19)
            sync.dma_start(out_d[64:128, :], out_t1[:]).then_inc(out_sem, 16)
            sync.wait_ge(out_sem, 32)
            sync.nop().then_inc(done_sem, 1)

        @block.gpsimd
        def _(gpsimd):
            # Semaphores persist across NEFF executions; clear them all at
            # the end (after every engine is provably done) so the kernel
            # is safe to run repeatedly.
            gpsimd.wait_ge(done_sem, 1)
            for s in [rs, *ks, *vs, dve_sem, pe_sem, out_sem, done_sem]:
                gpsimd.sem_clear(s)

        @block.scalar
        def _(scalar):
            scalar.wait_ge(dve_sem, 18)
            scalar.dma_start(out_d[0:64, :], out_t0[:]).then_inc(out_sem, 16)

        @block.vector
        def _(vector):
            vector.memset(acc[0][:], 0.0)
            vector.memset(acc[1][:], 0.0).then_inc(dve_sem, 1)
            vector.wait_ge(rs, 16)
            for m in range(N_MEGA):
                vector.wait_ge(ks[m], 16)
                for j in range(MEGA):
                    c = m * MEGA + j
                    vector.tensor_scalar_mul(
                        kts[:, c, :],
                        kt[:, c, :],
                        rho_t[:, c : c + 1],
                    ).then_inc(dve_sem, 1)
            vector.wait_ge(pe_sem, 16)
            vector.tensor_copy(out_t0[:], acc[0][:]).then_inc(dve_sem, 1)
            vector.wait_ge(pe_sem, 17)
            vector.tensor_copy(out_t1[:], acc[1][:]).then_inc(dve_sem, 1)

        @block.tensor
        def _(tensor):
            for c in range(CHUNKS):
                tensor.wait_ge(vs[c], 16)
                # memset + scales up to chunk c done
                tensor.wait_ge(dve_sem, c + 2)
                last = c == CHUNKS - 1
                # Last chunk: bank-0 heads first so ACT can evacuate bank 0
                # while bank-1 heads still run.
                for h in range(H):
                    g, hh = divmod(h, 8)
                    mm = tensor.matmul(
                        acc[g][:, hh * Dv : (hh + 1) * Dv],
                        kts[:, c, h * Dk : (h + 1) * Dk],
                        vt[:, c, h * Dv : (h + 1) * Dv],
                        start=False,
                        stop=last and (h == 7 or h == H - 1),
                        skip_group_check=True,
                    )
                    if (last and h == 7) or h == H - 1:
                        mm.then_inc(pe_sem, 1)

    return nc


def _get_nc():
    global _nc_cache
    if _nc_cache is None:
        _nc_cache = _build_nc()
    return _nc_cache


def _make_in_maps(keys, values, write_strengths):
    # One full-array bf16 cast, then per-core (chunk, p, f) -> (p, chunk, f)
    # transposes so each DMA partition line is contiguous in DRAM.
    kf = keys.reshape(B * S, FD).astype(BF16)
    vf = values.reshape(B * S, FD).astype(BF16)
    wf = np.asarray(write_strengths).reshape(B * S).astype(np.float32)
    in_maps = []
    for c in range(N_CORES):
        sl = slice(c * NS, (c + 1) * NS)
        in_maps.append(
            {
                "keys": np.ascontiguousarray(
                    kf[sl].reshape(CHUNKS, P, FD).transpose(1, 0, 2)
                ),
                "values": np.ascontiguousarray(
                    vf[sl].reshape(CHUNKS, P, FD).transpose(1, 0, 2)
                ),
                "rho": np.ascontiguousarray(wf[sl].reshape(CHUNKS, P).T),
            }
        )
    return in_maps


def _run(in_maps, **kwargs):
    from concourse.bass_utils import run_bass_kernel_spmd

    nc = _get_nc()
    return run_bass_kernel_spmd(nc, in_maps, core_ids=list(range(N_CORES)), **kwargs)


def _assemble(memory, results):
    parts = np.stack([r["delta"] for r in results], axis=0)  # (8, 128, 512)
    delta = parts.sum(axis=0, dtype=np.float64)  # (128, 512)
    # rows 0-63: heads 0-7 as [k, hh*64+v]; rows 64-127: heads 8-15
    lo = delta[0:64].reshape(Dk, 8, Dv)
    hi = delta[64:128].reshape(Dk, 8, Dv)
    delta_hkv = np.concatenate([lo, hi], axis=1).transpose(1, 0, 2)  # (H, Dk, Dv)
    out = DECAY * np.asarray(memory, dtype=np.float64) + delta_hkv
    return out.astype(np.float32)


def kernel(memory, keys, values, write_strengths):
    memory = np.asarray(memory, dtype=np.float32)
    keys = np.asarray(keys, dtype=np.float32)
    values = np.asarray(values, dtype=np.float32)
    write_strengths = np.asarray(write_strengths, dtype=np.float32)

    in_maps = _make_in_maps(keys, values, write_strengths)
    res = _run(in_maps)
    return _assemble(memory, res.results)


if __name__ == "__main__":
    rng = np.random.default_rng(0)
    mem = rng.standard_normal((H, Dk, Dv), dtype=np.float32)
    k = rng.standard_normal((B, S, H, Dk), dtype=np.float32)
    v = rng.standard_normal((B, S, H, Dv), dtype=np.float32)
    w = rng.random((B, S), dtype=np.float32)
    out = kernel(mem, k, v, w)
    ref = DECAY * mem + np.einsum(
        "bs,bshk,bshv->hkv", w.astype(np.float64), k.astype(np.float64), v.astype(np.float64)
    )
    err = np.abs(out - ref).max() / np.abs(ref).max()
    print("self-check rel err:", err)


# revision 26
# speedup vs baseline: 2.3350x; 1.3077x over previous
"""Trainium2 Bass kernel for the L1Writer scatter-memory problem.

Computes   out = 0.95 * memory + einsum('bs,bshk,bshv->hkv', rho, keys, values)

Strategy: data-parallel over the flattened (B*S)=16384 token axis, 2048 rows
per core.  Each core computes its partial delta
    delta_h = (sqrt(rho) K_h)^T (sqrt(rho) V_h)     (per head h)
as a chain of 128-row PE matmuls accumulating in PSUM.  The 8 partial
(H,Dk,Dv) deltas are summed on the host (tiny: 256 KB each) and added to
decay*memory there.

sqrt(rho) is folded into BOTH keys and values on the host and the results
are cast to fp8 e4m3 (PSUM accumulation stays fp32; fp8 products are exact
in fp32, so the device result matches a host simulation of the quantized
inputs).  Measured end-to-end rel err 7.2e-3 on the fixed reference inputs,
inside the 2e-2 gate with 2.8x margin.  fp8 quarters HBM traffic vs fp32 --
the kernel is memory-bound -- and enables 128-column weight loads (two
adjacent heads per LDWEIGHTS) to halve the PE's weight-load bandwidth floor.

Per-core kernel layout:
  - keys/values arrive as (128, 16, 1024) fp8, host-transposed so that
    token c*128+p lives at [p, c, :]: every DMA partition line is one
    contiguous run in DRAM.  Pieces are sized small at the start (so the
    PE starts early) and small at the end (so the post-DMA tail is one
    chunk's matmuls); every DMA gets its own semaphore -- a shared
    counting semaphore across DMAs is unsound (16 SDMA engines inc
    independently and drain at different rates).
  - Per chunk c and head pair q: one LDWEIGHTS of K[:, heads 2q,2q+1]
    (128 columns -- FWL-eligible) and one 128-wide matmul against
    V[:, heads 2q,2q+1] accumulating into acc[q//4][:, q%4, :, :]
    ([128, 4, 2, 64] per PSUM bank).  Rows 0-63 of block half 0 hold
    head 2q's delta; rows 64-127 of half 1 hold head 2q+1's; the other
    two quadrants are don't-care cross products.  All matmuls use
    start=False onto DVE-memset zeros (first write per element
    overwrites or accumulates onto zero -- correct for any stale
    has_written bits).
  - The last chunk runs bank-0 pairs first: DVE evacuates bank 0's two
    valid quadrant strips (partition-aligned copies) while PE finishes
    bank 1, ACT DMAs the bank-0 half of the output (its own HWDGE ring)
    while DVE copies bank 1, then sync DMAs the bank-1 half.  DRAM
    delta is (128, 512) fp32 = [k | 64+k][g][q][v]; host reassembles
    (H, Dk, Dv) with h = 2*(4g+q) (+1 for rows 64-127).
"""

import numpy as np
import ml_dtypes

F8 = ml_dtypes.float8_e4m3   # matches mybir.dt.float8e4 on this platform

DECAY = 0.95
B, S, H, Dk, Dv = 4, 4096, 16, 64, 64
N_CORES = 8
NS = (B * S) // N_CORES          # 2048 rows per core
P = 128                          # partitions
CHUNKS = NS // P                 # 16 contraction chunks of 128 rows
FD = H * Dk                      # 1024 features per row
NPAIR = H // 2                   # 8 head pairs per chunk

# (start_chunk, end_chunk) per DMA piece; small at the start for early PE
# start, small at the end for a one-chunk post-DMA tail.
K_PIECES = [(0, 2), (2, 8), (8, 16)]
V_PIECES = [(0, 2), (2, 5), (5, 8), (8, 11), (11, 13), (13, 14), (14, 15), (15, 16)]

_nc_cache = None


def _build_nc():
    from contextlib import ExitStack

    import concourse.bass as bass
    import concourse.mybir as mybir

    f32 = mybir.dt.float32
    f8 = mybir.dt.float8e4
    nc = bass.Bass()

    keys_d = nc.dram_tensor("keys", (P, CHUNKS, FD), f8, kind="ExternalInput")
    vals_d = nc.dram_tensor("values", (P, CHUNKS, FD), f8, kind="ExternalInput")
    out_d = nc.dram_tensor("delta", (P, 512), f32, kind="ExternalOutput")

    # chunk -> index of the key/value piece that carries it
    k_of = [next(i for i, (a, b) in enumerate(K_PIECES) if a <= c < b) for c in range(CHUNKS)]
    v_of = [next(i for i, (a, b) in enumerate(V_PIECES) if a <= c < b) for c in range(CHUNKS)]

    # Raw bass (no Tile); all waits are standalone sequencer wait_ge ops.
    #
    # Semaphore budget: ksem[3] + vsem[8] (one per DMA, exact completion) +
    # dve_sem (1 memset + 4 evac copies) + pe_sem (+1 per chunk 0..14, +1
    # after last chunk's bank-0 pairs, +1 after bank-1 pairs) + out_sem
    # (+16 per output half) + done_sem.
    with ExitStack() as ctx:
        kt = ctx.enter_context(nc.sbuf_tensor("kt", [P, CHUNKS, FD], f8))
        vt = ctx.enter_context(nc.sbuf_tensor("vt", [P, CHUNKS, FD], f8))
        # out_big[p, g, q, v]: bank-g pair-q quadrant strips
        out_big = ctx.enter_context(nc.sbuf_tensor("out_big", [P, 2, 4, Dv], f32))
        acc = [
            ctx.enter_context(nc.psum_tensor(f"acc{i}", [P, 4, 2, Dv], f32))
            for i in range(2)
        ]
        ksem = [ctx.enter_context(nc.semaphore(name=f"ks{i}")) for i in range(len(K_PIECES))]
        vsem = [ctx.enter_context(nc.semaphore(name=f"vs{i}")) for i in range(len(V_PIECES))]
        dve_sem = ctx.enter_context(nc.semaphore(name="dve_sem"))
        pe_sem = ctx.enter_context(nc.semaphore(name="pe_sem"))
        out_sem = ctx.enter_context(nc.semaphore(name="out_sem"))
        done_sem = ctx.enter_context(nc.semaphore(name="done_sem"))
        block = ctx.enter_context(nc.Block())

        @block.sync
        def _(sync):
            # Interleave key/value pieces so each chunk's pair arrives about
            # when PE needs it; ring is FIFO so issue order = stream order.
            emitted_k = 0
            for vi, (a, b) in enumerate(V_PIECES):
                # emit any key piece whose chunks start before this value
                # piece ends
                while emitted_k < len(K_PIECES) and K_PIECES[emitted_k][0] < b:
                    ka, kb = K_PIECES[emitted_k]
                    sync.dma_start(
                        kt[:, ka:kb, :], keys_d[:, ka:kb, :]
                    ).then_inc(ksem[emitted_k], 16)
                    emitted_k += 1
                sync.dma_start(vt[:, a:b, :], vals_d[:, a:b, :]).then_inc(
                    vsem[vi], 16
                )
            sync.wait_ge(dve_sem, 5)
            sync.dma_start(out_d[:, 256:512], out_big[:, 1, :, :]).then_inc(
                out_sem, 16
            )
            sync.wait_ge(out_sem, 32)
            sync.nop().then_inc(done_sem, 1)

        @block.gpsimd
        def _(gpsimd):
            # Semaphores persist across NEFF executions; clear them all at
            # the end (after every engine is provably done) so the kernel
            # is safe to run repeatedly.
            gpsimd.wait_ge(done_sem, 1)
            for s in [*ksem, *vsem, dve_sem, pe_sem, out_sem, done_sem]:
                gpsimd.sem_clear(s)

        @block.scalar
        def _(scalar):
            # bank-0 half of the output on ACT's own HWDGE ring, while DVE
            # still evacuates bank 1.  ACT does no datapath work (an ACT
            # copy feeding an ACT dma_start raced -- the sequencer-level
            # DMA trigger does not order against the in-flight datapath op).
            scalar.wait_ge(dve_sem, 3)
            scalar.dma_start(out_d[:, 0:256], out_big[:, 0, :, :]).then_inc(
                out_sem, 16
            )

        @block.vector
        def _(vector):
            vector.memset(acc[0][:], 0.0)
            vector.memset(acc[1][:], 0.0).then_inc(dve_sem, 1)
            # bank 0 strips while PE finishes bank 1 (different banks)
            vector.wait_ge(pe_sem, 16)
            vector.tensor_copy(out_big[0:64, 0, :, :], acc[0][0:64, :, 0, :]).then_inc(
                dve_sem, 1
            )
            vector.tensor_copy(
                out_big[64:128, 0, :, :], acc[0][64:128, :, 1, :]
            ).then_inc(dve_sem, 1)
            vector.wait_ge(pe_sem, 17)
            vector.tensor_copy(out_big[0:64, 1, :, :], acc[1][0:64, :, 0, :]).then_inc(
                dve_sem, 1
            )
            vector.tensor_copy(
                out_big[64:128, 1, :, :], acc[1][64:128, :, 1, :]
            ).then_inc(dve_sem, 1)

        @block.tensor
        def _(tensor):
            last_k = last_v = -1
            for c in range(CHUNKS):
                if k_of[c] != last_k:
                    last_k = k_of[c]
                    tensor.wait_ge(ksem[last_k], 16)
                if v_of[c] != last_v:
                    last_v = v_of[c]
                    tensor.wait_ge(vsem[last_v], 16)
                if c == 0:
                    tensor.wait_ge(dve_sem, 1)  # memsets done
                last = c == CHUNKS - 1
                for q in range(NPAIR):
                    g, qq = divmod(q, 4)
                    mm = tensor.matmul(
                        acc[g][:, qq, :, :],
                        kt[:, c, q * 128 : (q + 1) * 128],
                        vt[:, c, q * 128 : (q + 1) * 128],
                        start=False,
                        stop=last and (q == 3 or q == NPAIR - 1),
                        skip_group_check=True,
                    )
                    if (last and q == 3) or q == NPAIR - 1:
                        mm.then_inc(pe_sem, 1)

    return nc


def _get_nc():
    global _nc_cache
    if _nc_cache is None:
        _nc_cache = _build_nc()
    return _nc_cache


def _make_in_maps(keys, values, write_strengths):
    # Fold sqrt(rho) into both operands in fp32, one fp8 cast for the full
    # arrays, then per-core (chunk, p, f) -> (p, chunk, f) transposes so
    # each DMA partition line is contiguous in DRAM.
    sq = np.sqrt(np.asarray(write_strengths, dtype=np.float32)).reshape(B * S, 1)
    kq = (keys.reshape(B * S, FD) * sq).astype(F8)
    vq = (values.reshape(B * S, FD) * sq).astype(F8)
    in_maps = []
    for c in range(N_CORES):
        sl = slice(c * NS, (c + 1) * NS)
        in_maps.append(
            {
                "keys": np.ascontiguousarray(
                    kq[sl].reshape(CHUNKS, P, FD).transpose(1, 0, 2)
                ),
                "values": np.ascontiguousarray(
                    vq[sl].reshape(CHUNKS, P, FD).transpose(1, 0, 2)
                ),
            }
        )
    return in_maps


def _run(in_maps, **kwargs):
    from concourse.bass_utils import run_bass_kernel_spmd

    nc = _get_nc()
    return run_bass_kernel_spmd(nc, in_maps, core_ids=list(range(N_CORES)), **kwargs)


def _assemble(memory, results):
    parts = np.stack([r["delta"] for r in results], axis=0)  # (8, 128, 512)
    arr = parts.sum(axis=0, dtype=np.float64)  # (128, 512)
    blk = arr.reshape(128, 2, 4, Dv)
    delta_hkv = np.empty((H, Dk, Dv))
    for g in range(2):
        for q in range(4):
            delta_hkv[2 * (4 * g + q)] = blk[0:64, g, q, :]
            delta_hkv[2 * (4 * g + q) + 1] = blk[64:128, g, q, :]
    out = DECAY * np.asarray(memory, dtype=np.float64) + delta_hkv
    return out.astype(np.float32)


def kernel(memory, keys, values, write_strengths):
    memory = np.asarray(memory, dtype=np.float32)
    keys = np.asarray(keys, dtype=np.float32)
    values = np.asarray(values, dtype=np.float32)
    write_strengths = np.asarray(write_strengths, dtype=np.float32)

    in_maps = _make_in_maps(keys, values, write_strengths)
    res = _run(in_maps)
    return _assemble(memory, res.results)


if __name__ == "__main__":
    rng = np.random.default_rng(0)
    mem = rng.standard_normal((H, Dk, Dv), dtype=np.float32)
    k = rng.standard_normal((B, S, H, Dk), dtype=np.float32)
    v = rng.standard_normal((B, S, H, Dv), dtype=np.float32)
    w = rng.random((B, S), dtype=np.float32)
    out = kernel(mem, k, v, w)
    ref = DECAY * mem + np.einsum(
        "bs,bshk,bshv->hkv", w.astype(np.float64), k.astype(np.float64), v.astype(np.float64)
    )
    err = np.abs(out - ref).max() / np.abs(ref).max()
    print("self-check rel err:", err)


# revision 29
# speedup vs baseline: 2.4573x; 1.0524x over previous
"""Trainium2 Bass kernel for the L1Writer scatter-memory problem.

Computes   out = 0.95 * memory + einsum('bs,bshk,bshv->hkv', rho, keys, values)

Strategy: data-parallel over the flattened (B*S)=16384 token axis, 2048 rows
per core.  Each core computes its partial delta
    delta_h = (sqrt(rho) K_h)^T (sqrt(rho) V_h)     (per head h)
as a chain of 128-row PE matmuls accumulating in PSUM.  The 8 partial
(H,Dk,Dv) deltas are summed on the host (tiny: 256 KB each) and added to
decay*memory there.

sqrt(rho) is folded into BOTH keys and values on the host and the results
are cast to fp8 e4m3 (PSUM accumulation stays fp32; fp8 products are exact
in fp32, so the device result matches a host simulation of the quantized
inputs).  Measured end-to-end rel err 7.2e-3 on the fixed reference inputs,
inside the 2e-2 gate with 2.8x margin.  fp8 quarters HBM traffic vs fp32 --
the kernel is memory-bound -- and enables 128-column weight loads (two
adjacent heads per LDWEIGHTS) to halve the PE's weight-load bandwidth floor.

Per-core kernel layout:
  - keys/values arrive as (128, 16, 1024) fp8, host-transposed so that
    token c*128+p lives at [p, c, :]: every DMA partition line is one
    contiguous run in DRAM.  Pieces are sized small at the start (so the
    PE starts early) and small at the end (so the post-DMA tail is one
    chunk's matmuls); every DMA gets its own semaphore -- a shared
    counting semaphore across DMAs is unsound (16 SDMA engines inc
    independently and drain at different rates).
  - Per chunk c and head pair q: one LDWEIGHTS of K[:, heads 2q,2q+1]
    (128 columns -- FWL-eligible) and one 128-wide matmul against
    V[:, heads 2q,2q+1] accumulating into acc[q//4][:, q%4, :, :]
    ([128, 4, 2, 64] per PSUM bank).  Rows 0-63 of block half 0 hold
    head 2q's delta; rows 64-127 of half 1 hold head 2q+1's; the other
    two quadrants are don't-care cross products.  All matmuls use
    start=False onto DVE-memset zeros (first write per element
    overwrites or accumulates onto zero -- correct for any stale
    has_written bits).
  - The last chunk runs bank-0 pairs first: DVE evacuates bank 0's two
    valid quadrant strips (partition-aligned copies) while PE finishes
    bank 1, ACT DMAs the bank-0 half of the output (its own HWDGE ring)
    while DVE copies bank 1, then sync DMAs the bank-1 half.  DRAM
    delta is (128, 512) fp32 = [k | 64+k][g][q][v]; host reassembles
    (H, Dk, Dv) with h = 2*(4g+q) (+1 for rows 64-127).
"""

import numpy as np
import ml_dtypes

F8 = ml_dtypes.float8_e4m3   # matches mybir.dt.float8e4 on this platform

DECAY = 0.95
B, S, H, Dk, Dv = 4, 4096, 16, 64, 64
N_CORES = 8
NS = (B * S) // N_CORES          # 2048 rows per core
P = 128                          # partitions
CHUNKS = NS // P                 # 16 contraction chunks of 128 rows
FD = H * Dk                      # 1024 features per row
NPAIR = H // 2                   # 8 head pairs per chunk

# (start_chunk, end_chunk) per DMA piece; small at the start for early PE
# start, small at the end for a one-chunk post-DMA tail.  Keys stream on the
# ACT HWDGE ring and values on the SP ring with the same piece boundaries:
# the two descriptor generators run in parallel (one ring can't stay fed --
# generation costs ~0.6-1.1us per trigger) and chunk c's keys and values
# arrive together since both rings drain at the same rate.
K_PIECES = [(0, 2), (2, 6), (6, 10), (10, 13), (13, 15), (15, 16)]
V_PIECES = [(0, 2), (2, 6), (6, 10), (10, 13), (13, 15), (15, 16)]

_nc_cache = None


def _build_nc():
    from contextlib import ExitStack

    import concourse.bass as bass
    import concourse.mybir as mybir

    f32 = mybir.dt.float32
    f8 = mybir.dt.float8e4
    nc = bass.Bass()

    keys_d = nc.dram_tensor("keys", (P, CHUNKS, FD), f8, kind="ExternalInput")
    vals_d = nc.dram_tensor("values", (P, CHUNKS, FD), f8, kind="ExternalInput")
    out_d = nc.dram_tensor("delta", (P, 512), f32, kind="ExternalOutput")

    # chunk -> index of the key/value piece that carries it
    k_of = [next(i for i, (a, b) in enumerate(K_PIECES) if a <= c < b) for c in range(CHUNKS)]
    v_of = [next(i for i, (a, b) in enumerate(V_PIECES) if a <= c < b) for c in range(CHUNKS)]

    # Raw bass (no Tile); all waits are standalone sequencer wait_ge ops.
    #
    # Semaphore budget: ksem[3] + vsem[8] (one per DMA, exact completion) +
    # dve_sem (1 memset + 4 evac copies) + pe_sem (+1 per chunk 0..14, +1
    # after last chunk's bank-0 pairs, +1 after bank-1 pairs) + out_sem
    # (+16 per output half) + done_sem.
    with ExitStack() as ctx:
        kt = ctx.enter_context(nc.sbuf_tensor("kt", [P, CHUNKS, FD], f8))
        vt = ctx.enter_context(nc.sbuf_tensor("vt", [P, CHUNKS, FD], f8))
        # out_big[p, g, q, v]: bank-g pair-q quadrant strips
        out_big = ctx.enter_context(nc.sbuf_tensor("out_big", [P, 2, 4, Dv], f32))
        acc = [
            ctx.enter_context(nc.psum_tensor(f"acc{i}", [P, 4, 2, Dv], f32))
            for i in range(2)
        ]
        ksem = [ctx.enter_context(nc.semaphore(name=f"ks{i}")) for i in range(len(K_PIECES))]
        vsem = [ctx.enter_context(nc.semaphore(name=f"vs{i}")) for i in range(len(V_PIECES))]
        dve_sem = ctx.enter_context(nc.semaphore(name="dve_sem"))
        pe_sem = ctx.enter_context(nc.semaphore(name="pe_sem"))
        out_sem = ctx.enter_context(nc.semaphore(name="out_sem"))
        done_sem = ctx.enter_context(nc.semaphore(name="done_sem"))
        block = ctx.enter_context(nc.Block())

        @block.sync
        def _(sync):
            for vi, (a, b) in enumerate(V_PIECES):
                sync.dma_start(vt[:, a:b, :], vals_d[:, a:b, :]).then_inc(
                    vsem[vi], 16
                )
            sync.wait_ge(dve_sem, 5)
            sync.dma_start(out_d[:, 256:512], out_big[:, 1, :, :]).then_inc(
                out_sem, 16
            )
            sync.wait_ge(out_sem, 32)
            sync.nop().then_inc(done_sem, 1)

        @block.gpsimd
        def _(gpsimd):
            # Semaphores persist across NEFF executions; clear them all at
            # the end (after every engine is provably done) so the kernel
            # is safe to run repeatedly.
            gpsimd.wait_ge(done_sem, 1)
            for s in [*ksem, *vsem, dve_sem, pe_sem, out_sem, done_sem]:
                gpsimd.sem_clear(s)

        @block.scalar
        def _(scalar):
            # Keys stream on ACT's HWDGE ring, in parallel with values on SP.
            for ki, (a, b) in enumerate(K_PIECES):
                scalar.dma_start(kt[:, a:b, :], keys_d[:, a:b, :]).then_inc(
                    ksem[ki], 16
                )
            # Then the bank-0 half of the output, while DVE still evacuates
            # bank 1.  ACT does no datapath work (an ACT copy feeding an ACT
            # dma_start raced -- the sequencer-level DMA trigger does not
            # order against the in-flight datapath op).
            scalar.wait_ge(dve_sem, 3)
            scalar.dma_start(out_d[:, 0:256], out_big[:, 0, :, :]).then_inc(
                out_sem, 16
            )

        @block.vector
        def _(vector):
            vector.memset(acc[0][:], 0.0)
            vector.memset(acc[1][:], 0.0).then_inc(dve_sem, 1)
            # bank 0 strips while PE finishes bank 1 (different banks)
            vector.wait_ge(pe_sem, 16)
            vector.tensor_copy(out_big[0:64, 0, :, :], acc[0][0:64, :, 0, :]).then_inc(
                dve_sem, 1
            )
            vector.tensor_copy(
                out_big[64:128, 0, :, :], acc[0][64:128, :, 1, :]
            ).then_inc(dve_sem, 1)
            vector.wait_ge(pe_sem, 17)
            vector.tensor_copy(out_big[0:64, 1, :, :], acc[1][0:64, :, 0, :]).then_inc(
                dve_sem, 1
            )
            vector.tensor_copy(
                out_big[64:128, 1, :, :], acc[1][64:128, :, 1, :]
            ).then_inc(dve_sem, 1)

        @block.tensor
        def _(tensor):
            last_k = last_v = -1
            for c in range(CHUNKS):
                if k_of[c] != last_k:
                    last_k = k_of[c]
                    tensor.wait_ge(ksem[last_k], 16)
                if v_of[c] != last_v:
                    last_v = v_of[c]
                    tensor.wait_ge(vsem[last_v], 16)
                if c == 0:
                    tensor.wait_ge(dve_sem, 1)  # memsets done
                last = c == CHUNKS - 1
                for q in range(NPAIR):
                    g, qq = divmod(q, 4)
                    mm = tensor.matmul(
                        acc[g][:, qq, :, :],
                        kt[:, c, q * 128 : (q + 1) * 128],
                        vt[:, c, q * 128 : (q + 1) * 128],
                        start=False,
                        stop=last and (q == 3 or q == NPAIR - 1),
                        skip_group_check=True,
                    )
                    if (last and q == 3) or q == NPAIR - 1:
                        mm.then_inc(pe_sem, 1)

    return nc


def _get_nc():
    global _nc_cache
    if _nc_cache is None:
        _nc_cache = _build_nc()
    return _nc_cache


def _make_in_maps(keys, values, write_strengths):
    # Fold sqrt(rho) into both operands in fp32, one fp8 cast for the full
    # arrays, then per-core (chunk, p, f) -> (p, chunk, f) transposes so
    # each DMA partition line is contiguous in DRAM.
    sq = np.sqrt(np.asarray(write_strengths, dtype=np.float32)).reshape(B * S, 1)
    kq = (keys.reshape(B * S, FD) * sq).astype(F8)
    vq = (values.reshape(B * S, FD) * sq).astype(F8)
    in_maps = []
    for c in range(N_CORES):
        sl = slice(c * NS, (c + 1) * NS)
        in_maps.append(
            {
                "keys": np.ascontiguousarray(
                    kq[sl].reshape(CHUNKS, P, FD).transpose(1, 0, 2)
                ),
                "values": np.ascontiguousarray(
                    vq[sl].reshape(CHUNKS, P, FD).transpose(1, 0, 2)
                ),
            }
        )
    return in_maps


def _run(in_maps, **kwargs):
    from concourse.bass_utils import run_bass_kernel_spmd

    nc = _get_nc()
    return run_bass_kernel_spmd(nc, in_maps, core_ids=list(range(N_CORES)), **kwargs)


def _assemble(memory, results):
    parts = np.stack([r["delta"] for r in results], axis=0)  # (8, 128, 512)
    arr = parts.sum(axis=0, dtype=np.float64)  # (128, 512)
    blk = arr.reshape(128, 2, 4, Dv)
    delta_hkv = np.empty((H, Dk, Dv))
    for g in range(2):
        for q in range(4):
            delta_hkv[2 * (4 * g + q)] = blk[0:64, g, q, :]
            delta_hkv[2 * (4 * g + q) + 1] = blk[64:128, g, q, :]
    out = DECAY * np.asarray(memory, dtype=np.float64) + delta_hkv
    return out.astype(np.float32)


def kernel(memory, keys, values, write_strengths):
    memory = np.asarray(memory, dtype=np.float32)
    keys = np.asarray(keys, dtype=np.float32)
    values = np.asarray(values, dtype=np.float32)
    write_strengths = np.asarray(write_strengths, dtype=np.float32)

    in_maps = _make_in_maps(keys, values, write_strengths)
    res = _run(in_maps)
    return _assemble(memory, res.results)


if __name__ == "__main__":
    rng = np.random.default_rng(0)
    mem = rng.standard_normal((H, Dk, Dv), dtype=np.float32)
    k = rng.standard_normal((B, S, H, Dk), dtype=np.float32)
    v = rng.standard_normal((B, S, H, Dv), dtype=np.float32)
    w = rng.random((B, S), dtype=np.float32)
    out = kernel(mem, k, v, w)
    ref = DECAY * mem + np.einsum(
        "bs,bshk,bshv->hkv", w.astype(np.float64), k.astype(np.float64), v.astype(np.float64)
    )
    err = np.abs(out - ref).max() / np.abs(ref).max()
    print("self-check rel err:", err)
